# revision 47
# baseline (speedup 1.0000x reference)
"""Trainium2 Bass kernel for the box-ranking depth loss (v5).

Structure (vs the v2 kernel):
  - ONE interleaved fp16 slab I[p, 2c+s] = (+d, -d)[s] at column c replaces
    the separate dsn/dsh copies.  The five sliding-max table levels each
    process BOTH the max side and the (negated) min side in a single packed
    stride-1 fp16 tensor_tensor (DVE 2x mode), and every box needs ONE
    fused lookup (4-dim strided TensorReduce that keeps the interleave
    pair) instead of two; the pair lands directly in BLOCK layout
    (cols 0:32 = max, 32:64 = -min) via a strided output AP.  Level 1
    chases the four input-DMA chunks; L32 is restricted to the boxes'
    column span (bboxes are compile-time constants).
  - Cross-partition combines use gpsimd.partition_all_reduce, removing the
    128x128 identity matrix (and its 64KB DMA) entirely.  (Pool cannot run
    max tensor_tensor or touch PSUM, so the tables stay on DVE.)
  - Sums/sumsq ride the PE engine via the column-transposed dsT slab and
    per-chunk column indicators; ACT squares dsT.
  - Three AllReduce collectives: sums (add), core-local global (max|-min)
    range (max), and per-box (max|-min) stats (max).  AllReduce means the
    readbacks are [1,N] rows with no per-lane re-reduction.  The wave-1
    machinery lives on the ACT DMA queue and finishes during the lookups;
    the wave-2 tail chain (ship -> collective -> readback -> out) runs on
    the SP queue with DVE-only final math.

Sharding: rows (H) split 8 ways -> each core holds 128 rows.  Every core
computes the final 3-float result redundantly; the host reads core 0.
"""

import numpy as np

H, W, T, NCORES = 1024, 2048, 32, 8
R = H // NCORES  # 128 rows per core
BIG = 1e30
RATIO = 1.0
NCHUNK = 16  # 2048 cols / 128

NSLOT = 2 * W                      # interleaved slots
CH1 = 1088                         # first DMA chunk of I (slots)
DINH_W = NSLOT + W + 32 * NCHUNK   # I | dsT | cold
CST_W = 228


def _win_params(x1, x2):
    """Width-32 windows covering [x1, x2): n windows, two interleaved
    arithmetic progressions offset by s1 (s1 == 0 -> single AP)."""
    q = (x2 - x1) - 32
    n = q // 32 + 1
    s1 = q - 32 * (n - 1)
    return n, s1


def _build_program(bboxes, single_core=False, reps=1, mock_cc=False):
    import concourse.bacc as bacc
    import concourse.mybir as mybir
    import concourse.tile as tile
    import concourse.bass_isa as bass_isa
    from concourse.ap import AP
    from concourse.alu_op_type import AluOpType as alu

    f32 = mybir.dt.float32
    f16 = mybir.dt.float16
    X = mybir.AxisListType.X
    XY = mybir.AxisListType.XY
    AF = mybir.ActivationFunctionType
    RO = bass_isa.ReduceOp

    x1s, x2s = bboxes[:, 0], bboxes[:, 2]
    # L32 is only read by the box lookups: restrict it to the box x-span
    lo32 = 2 * int(x1s.min())
    hi32 = min(2 * (int(x2s.max()) - 32) + 2 + 62, NSLOT - 62)

    nc = bacc.Bacc("TRN2", target_bir_lowering=False, debug=False,
                   num_devices=1 if single_core else NCORES)

    dinH = nc.dram_tensor("dinH", [R, DINH_W], f16, kind="ExternalInput").ap()
    cst = nc.dram_tensor("cst", [128, CST_W], f32, kind="ExternalInput").ap()
    out = nc.dram_tensor("out", [3], f32, kind="ExternalOutput").ap()

    def sb(name, shape, dt=f32):
        return nc.alloc_sbuf_tensor(name, shape, dt).ap()

    I0 = sb("I0", [R, NSLOT], f16)
    L2 = sb("L2", [R, NSLOT], f16)
    L4 = sb("L4", [R, NSLOT], f16)
    L8 = sb("L8", [R, NSLOT], f16)
    L16 = sb("L16", [R, NSLOT], f16)
    L32 = sb("L32", [R, NSLOT], f16)
    dsTC = sb("dsTC", [R, W + 32 * NCHUNK], f16)   # dsT | cold
    dsq = sb("dsq", [R, W], f16)
    cstS = sb("cstS", [128, CST_W])
    gfix = sb("gfix", [R, 2])            # per-row (max | -min)
    ggP = sb("ggP", [128, 2])            # par-reduced (max | -min)
    rmm = sb("rmm", [R, 2 * T])          # lookup out: (max | -min) pairs
    stk = sb("stk", [R, 2 * T])
    stkP = sb("stkP", [128, 2 * T])
    rrs = sb("rrs", [R, 2 * T])
    svS = sb("svS", [2 * T, 1])
    sv2 = sb("sv2", [2 * T, 1])
    ggm = sb("ggm", [1, 2])
    aden = sb("aden", [1, 1])
    arecip = sb("arecip", [1, 1])
    acolS = sb("acolS", [T, 1])
    meanv = sb("meanv", [T, 1])
    m2sv = sb("m2sv", [T, 1])
    varv = sb("varv", [T, 1])
    stdv = sb("stdv", [T, 1])
    meanTS = sb("meanTS", [1, T])
    qm = sb("qm", [T, T])
    t2m = sb("t2m", [T, T])
    t3m = sb("t3m", [T, T])
    raccv = sb("raccv", [T, 1])
    redM = sb("redM", [2 * T, 1])
    rngv = sb("rngv", [T, 1])
    rinv = sb("rinv", [T, 1])
    srv = sb("srv", [T, 1])
    dummy = sb("dmy0", [1, 8])
    out3 = sb("out3", [1, 3])

    # const views
    ident32C = cstS[0:T, 0:T]
    gmatC = cstS[0:T, 32:64]
    cntinvC = cstS[0:T, 64:65]
    cm1invC = cstS[0:T, 65:66]
    onesC = cstS[:, 66:67]          # all-128-rows ones; [0:T] view for T
    onesrowC = cstS[0:1, 68:68 + T]
    rinfC = cstS[:, 100:164]        # interleaved row mask, 0 / -BIG
    rind2C = cstS[0:128, 164:228]

    def box_lookup(t, x1, x2):
        """One fused (max | -min) lookup for box t: width-32 windows over
        [x1, x2) on the interleaved L32 table; keeps the pair dim.  Output
        lands in BLOCK layout: col t = max side, col T+t = -min side."""
        n, s1 = _win_params(x1, x2)
        base = L32[:, 0:1]
        ppair = list(base.ap[0])
        if s1 == 0:
            v = AP(base.tensor, base.offset + 2 * x1,
                   [ppair, [1, 2], [64, n]])
            ax = X
        else:
            v = AP(base.tensor, base.offset + 2 * x1,
                   [ppair, [1, 2], [2 * s1, 2], [64, n]])
            ax = XY
        ob = rmm[:, 0:1]
        o = AP(ob.tensor, ob.offset + t, [list(ob.ap[0]), [T, 2]])
        nc.vector.tensor_reduce(o, v, ax, alu.max)

    with tile.TileContext(nc) as tc:
        with tc.tile_pool(name="psum", bufs=1, space="PSUM") as pp, \
                tc.tile_pool(name="dram", bufs=1, space="DRAM") as dram:
            rowboth = pp.tile([R, 2 * T], f32, name="rowboth")
            psum_s = pp.tile([2 * T, 1], f32, name="psum_s")
            meanT_p = pp.tile([1, T], f32, name="meanT_p")
            mr_p = pp.tile([T, T], f32, name="mr_p")
            pl2 = pp.tile([1, 3], f32, name="pl2")

            cstatS = dram.tile([1, 2 * T], f32, name="cstatS")
            cgathS = dram.tile([1, 2 * T], f32, name="cgathS")
            cstatX = dram.tile([1, 2 * T], f32, name="cstatX")
            cgathX = dram.tile([1, 2 * T], f32, name="cgathX")
            cstatG = dram.tile([1, 2], f32, name="cstatG")
            cgathG = dram.tile([1, 2], f32, name="cgathG")

            def allred(cstat, cgath, op, q):
                if single_core or mock_cc:
                    q.dma_start(out=cgath[:], in_=cstat[:])
                else:
                    nc.gpsimd.collective_compute(
                        "AllReduce", op,
                        replica_groups=[list(range(NCORES))],
                        ins=[cstat[:]], outs=[cgath[:]])

            for _rep in range(reps):
                # ---- input loads: I in 5 chunks (2 queues, small first
                # chunk for an early DVE start), dsT+cold, cst ----
                cb = [0, 512, 1024, 2048, 3072, NSLOT]
                qs = [nc.sync, nc.scalar, nc.sync, nc.scalar, nc.sync]
                for i in range(5):
                    qs[i].dma_start(out=I0[:, cb[i]:cb[i + 1]],
                                    in_=dinH[:, cb[i]:cb[i + 1]])
                nc.scalar.dma_start(out=dsTC[:], in_=dinH[:, NSLOT:DINH_W])
                nc.sync.dma_start(out=cstS[:], in_=cst[:])

                # ---- ACT function-table preloads (hidden under input DMA) --
                nc.vector.memset(dummy[0:1, 0:1], 0.0)
                nc.scalar.activation(dummy[0:1, 1:2], dummy[0:1, 0:1],
                                     AF.Square)
                nc.scalar.activation(dummy[0:1, 2:3], dummy[0:1, 0:1],
                                     AF.Sqrt)
                nc.scalar.activation(dummy[0:1, 3:4], dummy[0:1, 0:1],
                                     AF.Relu)
                nc.scalar.copy(dummy[0:1, 4:5], dummy[0:1, 0:1])

                # ---- interleaved sliding max tables (all DVE).
                # Level-2 pieces chase the input chunks; level-4 pieces are
                # interleaved into the chase so DVE never idles waiting for
                # a DMA chunk (the per-engine queue executes in issue order).
                def l2p(lo, hi):
                    nc.vector.tensor_tensor(L2[:, lo:hi], I0[:, lo:hi],
                                            I0[:, lo + 2:hi + 2], alu.max)

                def l4p(lo, hi):
                    nc.vector.tensor_tensor(L4[:, lo:hi], L2[:, lo:hi],
                                            L2[:, lo + 4:hi + 4], alu.max)

                l2p(0, cb[1] - 2)
                l2p(cb[1] - 2, cb[2] - 2)
                l2p(cb[2] - 2, cb[3] - 2)
                l2p(cb[3] - 2, cb[4] - 2)
                l2p(cb[4] - 2, NSLOT - 2)
                l4p(0, cb[3] - 6)
                l4p(cb[3] - 6, NSLOT - 6)
                nc.vector.tensor_tensor(L8[:, 0:NSLOT - 14],
                                        L4[:, 0:NSLOT - 14],
                                        L4[:, 8:NSLOT - 6], alu.max)
                nc.vector.tensor_tensor(L16[:, 0:NSLOT - 30],
                                        L8[:, 0:NSLOT - 30],
                                        L8[:, 16:NSLOT - 14], alu.max)
                # global per-row (max | -min) from L16 (early, so the whole
                # wave-1 chain clears HWDGE during the lookups)
                gv = AP(L16.tensor, L16.offset, [list(L16.ap[0]),
                                                 [1, 2], [32, 128]])
                nc.vector.tensor_reduce(gfix[:], gv, X, alu.max)
                nc.vector.tensor_tensor(L32[:, lo32:hi32],
                                        L16[:, lo32:hi32],
                                        L16[:, lo32 + 32:hi32 + 32], alu.max)

                # ---- PE sums path (square on ACT, matmuls on PE) ----
                nc.scalar.square(dsq[:], dsTC[:, 0:W])
                for k in range(NCHUNK):
                    nc.tensor.matmul(rowboth[:, 0:T],
                                     dsTC[:, 128 * k:128 * (k + 1)],
                                     dsTC[:, W + 32 * k:W + 32 * (k + 1)],
                                     start=(k == 0), stop=(k == NCHUNK - 1))
                for k in range(NCHUNK):
                    nc.tensor.matmul(rowboth[:, T:2 * T],
                                     dsq[:, 128 * k:128 * (k + 1)],
                                     dsTC[:, W + 32 * k:W + 32 * (k + 1)],
                                     start=(k == 0), stop=(k == NCHUNK - 1))

                # mask rows (one fused op), reduce to per-box sums on PE
                nc.vector.tensor_tensor(rrs[:], rowboth[:], rind2C, alu.mult)
                nc.tensor.matmul(psum_s[:], rrs[:], onesC,
                                 start=True, stop=True)
                nc.scalar.copy(svS[:], psum_s[:])
                # core-local global (max | -min) across partitions on Pool
                nc.gpsimd.partition_all_reduce(ggP[:], gfix[:], 128, RO.max)
                nc.scalar.dma_start(out=cstatS[0:1, :], in_=svS[:])
                nc.scalar.dma_start(out=cstatG[0:1, :], in_=ggP[0:1, 0:2])
                allred(cstatS, cgathS, alu.add, nc.scalar)
                allred(cstatG, cgathG, alu.max, nc.scalar)

                # ---- fused per-box lookups ----
                for t in range(T):
                    box_lookup(t, int(x1s[t]), int(x2s[t]))
                # row mask (DVE, right after the last lookup) + Pool reduce,
                # feeding the wave-2 tail chain on the SP queue
                nc.vector.tensor_tensor(stk[:], rmm[:], rinfC, alu.add)
                nc.gpsimd.partition_all_reduce(stkP[:], stk[:], 128, RO.max)
                nc.sync.dma_start(out=cstatX[0:1, :], in_=stkP[0:1, 0:2 * T])
                allred(cstatX, cgathX, alu.max, nc.sync)

                # ---- wave-1 readback + mean / loss_acc pipeline ----
                nc.scalar.dma_start(out=sv2[:],
                                    in_=cgathS[0:1, :].transpose([1, 0]))
                nc.scalar.dma_start(out=ggm[:], in_=cgathG[0:1, :])
                nc.vector.tensor_tensor(aden[:], ggm[:, 0:1], ggm[:, 1:2],
                                        alu.add)
                nc.vector.reciprocal(arecip[:], aden[:])
                nc.gpsimd.partition_broadcast(acolS[:], arecip[:])
                nc.vector.tensor_scalar_mul(meanv[:], sv2[0:T, 0:1], cntinvC)
                nc.vector.tensor_scalar_mul(m2sv[:], sv2[0:T, 0:1], meanv[:])
                nc.vector.tensor_scalar(varv[:], sv2[T:2 * T, 0:1], m2sv[:],
                                        cm1invC, alu.subtract, alu.mult)
                nc.scalar.sqrt(stdv[:], varv[:])
                nc.tensor.transpose(meanT_p[:], meanv[:], ident32C)
                nc.scalar.copy(meanTS[:], meanT_p[:])
                nc.tensor.matmul(mr_p[:], onesrowC, meanTS[:],
                                 start=True, stop=True)
                nc.vector.tensor_scalar(qm[:], mr_p[:], meanv[:], acolS[:],
                                        alu.subtract, alu.mult)
                nc.vector.tensor_tensor(t2m[:], gmatC, qm[:], alu.subtract)
                nc.scalar.activation(t3m[:], t2m[:], AF.Relu,
                                     accum_out=raccv[:])
                nc.tensor.matmul(pl2[:, 0:1], raccv[:], cstS[0:T, 66:67],
                                 start=True, stop=True)

                # ---- wave-2 readback + final (SP queue, DVE math) ----
                # redM rows 0:32 = global max_t, rows 32:64 = global -min_t
                # (block layout straight from the lookups; AllReduce'd)
                nc.sync.dma_start(out=redM[:],
                                  in_=cgathX[0:1, :].transpose([1, 0]))
                nc.vector.tensor_scalar_add(rngv[:], redM[T:2 * T, 0:1],
                                            redM[0:T, 0:1])
                nc.vector.reciprocal(rinv[:], rngv[:])
                nc.vector.tensor_tensor(srv[:], stdv[:], rinv[:], alu.mult)
                nc.tensor.matmul(pl2[:, 1:2], srv[:], cstS[0:T, 66:67],
                                 start=True, stop=True)
                nc.vector.tensor_scalar(out3[:, 0:2], pl2[:, 0:2], 0.0, 0.0,
                                        alu.add, alu.add,
                                        accum_out=out3[:, 2:3])
                nc.sync.dma_start(out=out[:], in_=out3[0:1, 0:3])

    nc.compile()
    return nc


def kernel(d_pred, bboxes, _trace=False):
    from concourse.bass_utils import run_bass_kernel_spmd

    d_pred = np.asarray(d_pred, dtype=np.float32)
    bboxes = np.asarray(bboxes, dtype=np.int32)
    depth = d_pred[0, 0]
    x1, y1, x2, y2 = (bboxes[:, i].astype(np.int64) for i in range(4))

    cnt = ((x2 - x1) * (y2 - y1)).astype(np.float64)
    cntinv = (1.0 / cnt).astype(np.float32)
    cm1inv = (1.0 / (cnt - 1.0)).astype(np.float32)

    ii = np.arange(T)[:, None]
    jj = np.arange(T)[None, :]
    gmat = np.where(jj > ii, (jj - ii) / float(T), -BIG).astype(np.float32)

    rows = np.arange(H)
    rind_full = ((rows[:, None] >= y1[None, :])
                 & (rows[:, None] < y2[None, :])).astype(np.float32)

    cols = np.arange(W)
    colind_full = ((cols[:, None] >= x1[None, :])
                   & (cols[:, None] < x2[None, :])).astype(np.float16)
    cold_h = colind_full.reshape(NCHUNK, 128, T).transpose(1, 0, 2) \
        .reshape(128, NCHUNK * T)

    in_maps = []
    for c in range(NCORES):
        dloc = depth[c * R:(c + 1) * R]                       # [128, 2048]
        ri = rind_full[c * R:(c + 1) * R]                     # [128, 32]
        rinf = np.where(ri > 0, 0.0, -BIG).astype(np.float32)

        cstc = np.zeros((128, CST_W), np.float32)
        cstc[0:T, 0:T] = np.eye(T, dtype=np.float32)
        cstc[0:T, 32:64] = gmat
        cstc[0:T, 64] = cntinv
        cstc[0:T, 65] = cm1inv
        cstc[:, 66] = 1.0
        cstc[0, 68:68 + T] = 1.0
        # block row mask: cols 0:32 max side, 32:64 -min side (same mask)
        cstc[:, 100:132] = rinf
        cstc[:, 132:164] = rinf
        cstc[:, 164:196] = ri
        cstc[:, 196:228] = ri

        dsT_h = dloc.T.reshape(NCHUNK, 128, R).transpose(1, 0, 2) \
            .reshape(128, W).astype(np.float16)

        dinH = np.empty((R, DINH_W), np.float16)
        dinH[:, 0:NSLOT:2] = dloc.astype(np.float16)
        dinH[:, 1:NSLOT:2] = (-dloc).astype(np.float16)
        dinH[:, NSLOT:NSLOT + W] = dsT_h
        dinH[:, NSLOT + W:DINH_W] = cold_h
        in_maps.append({"dinH": dinH, "cst": cstc})

    nc = _build_program(bboxes)
    res = run_bass_kernel_spmd(nc, in_maps, list(range(NCORES)),
                               trace=_trace)
    o = res.results[0]["out"].astype(np.float32)
    outs = (o[0:1].copy(), o[1:2].copy(), o[2:3].copy())
    if _trace:
        return outs, res
    return outs


# revision 48
# speedup vs baseline: 1.0244x; 1.0244x over previous
"""Trainium2 Bass kernel for the box-ranking depth loss (v5).

Structure (vs the v2 kernel):
  - ONE interleaved fp16 slab I[p, 2c+s] = (+d, -d)[s] at column c replaces
    the separate dsn/dsh copies.  The five sliding-max table levels each
    process BOTH the max side and the (negated) min side in a single packed
    stride-1 fp16 tensor_tensor (DVE 2x mode), and every box needs ONE
    fused lookup (4-dim strided TensorReduce that keeps the interleave
    pair) instead of two; the pair lands directly in BLOCK layout
    (cols 0:32 = max, 32:64 = -min) via a strided output AP.  Level 1
    chases the four input-DMA chunks; L32 is restricted to the boxes'
    column span (bboxes are compile-time constants).
  - Cross-partition combines use gpsimd.partition_all_reduce, removing the
    128x128 identity matrix (and its 64KB DMA) entirely.  (Pool cannot run
    max tensor_tensor or touch PSUM, so the tables stay on DVE.)
  - Sums/sumsq ride the PE engine via the column-transposed dsT slab and
    per-chunk column indicators; ACT squares dsT.
  - Three AllReduce collectives: sums (add), core-local global (max|-min)
    range (max), and per-box (max|-min) stats (max).  AllReduce means the
    readbacks are [1,N] rows with no per-lane re-reduction.  The wave-1
    machinery lives on the ACT DMA queue and finishes during the lookups;
    the wave-2 tail chain (ship -> collective -> readback -> out) runs on
    the SP queue with DVE-only final math.

Sharding: rows (H) split 8 ways -> each core holds 128 rows.  Every core
computes the final 3-float result redundantly; the host reads core 0.
"""

import numpy as np

H, W, T, NCORES = 1024, 2048, 32, 8
R = H // NCORES  # 128 rows per core
BIG = 1e30
RATIO = 1.0
NCHUNK = 16  # 2048 cols / 128

NSLOT = 2 * W                      # interleaved slots
CH1 = 1088                         # first DMA chunk of I (slots)
DINH_W = NSLOT + W + 32 * NCHUNK   # I | dsT | cold
CST_W = 228


def _win_params(x1, x2):
    """Width-32 windows covering [x1, x2): n windows, two interleaved
    arithmetic progressions offset by s1 (s1 == 0 -> single AP)."""
    q = (x2 - x1) - 32
    n = q // 32 + 1
    s1 = q - 32 * (n - 1)
    return n, s1


def _build_program(bboxes, single_core=False, reps=1, mock_cc=False):
    import concourse.bacc as bacc
    import concourse.mybir as mybir
    import concourse.tile as tile
    import concourse.bass_isa as bass_isa
    from concourse.ap import AP
    from concourse.alu_op_type import AluOpType as alu

    f32 = mybir.dt.float32
    f16 = mybir.dt.float16
    X = mybir.AxisListType.X
    XY = mybir.AxisListType.XY
    AF = mybir.ActivationFunctionType
    RO = bass_isa.ReduceOp

    x1s, x2s = bboxes[:, 0], bboxes[:, 2]
    # L32 is only read by the box lookups: restrict it to the box x-span
    lo32 = 2 * int(x1s.min())
    hi32 = min(2 * (int(x2s.max()) - 32) + 2 + 62, NSLOT - 62)

    nc = bacc.Bacc("TRN2", target_bir_lowering=False, debug=False,
                   num_devices=1 if single_core else NCORES)

    dinH = nc.dram_tensor("dinH", [R, DINH_W], f16, kind="ExternalInput").ap()
    cst = nc.dram_tensor("cst", [128, CST_W], f32, kind="ExternalInput").ap()
    out = nc.dram_tensor("out", [3], f32, kind="ExternalOutput").ap()

    def sb(name, shape, dt=f32):
        return nc.alloc_sbuf_tensor(name, shape, dt).ap()

    I0 = sb("I0", [R, NSLOT], f16)
    L2 = sb("L2", [R, NSLOT], f16)
    L4 = sb("L4", [R, NSLOT], f16)
    L8 = sb("L8", [R, NSLOT], f16)
    L16 = sb("L16", [R, NSLOT], f16)
    L32 = sb("L32", [R, NSLOT], f16)
    dsTC = sb("dsTC", [R, W + 32 * NCHUNK], f16)   # dsT | cold
    dsq = sb("dsq", [R, W], f16)
    cstS = sb("cstS", [128, CST_W])
    gfix = sb("gfix", [R, 2])            # per-row (max | -min)
    ggP = sb("ggP", [128, 2])            # par-reduced (max | -min)
    rmm = sb("rmm", [R, 2 * T])          # lookup out: (max | -min) pairs
    stk = sb("stk", [R, 2 * T])
    stkP = sb("stkP", [128, 2 * T])
    rrs = sb("rrs", [R, 2 * T])
    svS = sb("svS", [2 * T, 1])
    sv2 = sb("sv2", [2 * T, 1])
    ggm = sb("ggm", [1, 2])
    aden = sb("aden", [1, 1])
    arecip = sb("arecip", [1, 1])
    acolS = sb("acolS", [T, 1])
    meanv = sb("meanv", [T, 1])
    m2sv = sb("m2sv", [T, 1])
    varv = sb("varv", [T, 1])
    stdv = sb("stdv", [T, 1])
    meanTS = sb("meanTS", [1, T])
    qm = sb("qm", [T, T])
    t2m = sb("t2m", [T, T])
    t3m = sb("t3m", [T, T])
    raccv = sb("raccv", [T, 1])
    redM = sb("redM", [2 * T, 1])
    rngv = sb("rngv", [T, 1])
    rinv = sb("rinv", [T, 1])
    srv = sb("srv", [T, 1])
    dummy = sb("dmy0", [1, 8])
    out3 = sb("out3", [1, 3])

    # const views
    ident32C = cstS[0:T, 0:T]
    gmatC = cstS[0:T, 32:64]
    cntinvC = cstS[0:T, 64:65]
    cm1invC = cstS[0:T, 65:66]
    onesC = cstS[:, 66:67]          # all-128-rows ones; [0:T] view for T
    onesrowC = cstS[0:1, 68:68 + T]
    rinfC = cstS[:, 100:164]        # interleaved row mask, 0 / -BIG
    rind2C = cstS[0:128, 164:228]

    def box_lookup(t, x1, x2):
        """One fused (max | -min) lookup for box t: width-32 windows over
        [x1, x2) on the interleaved L32 table; keeps the pair dim.  Output
        lands in BLOCK layout: col t = max side, col T+t = -min side."""
        n, s1 = _win_params(x1, x2)
        base = L32[:, 0:1]
        ppair = list(base.ap[0])
        if s1 == 0:
            v = AP(base.tensor, base.offset + 2 * x1,
                   [ppair, [1, 2], [64, n]])
            ax = X
        else:
            v = AP(base.tensor, base.offset + 2 * x1,
                   [ppair, [1, 2], [2 * s1, 2], [64, n]])
            ax = XY
        ob = rmm[:, 0:1]
        o = AP(ob.tensor, ob.offset + t, [list(ob.ap[0]), [T, 2]])
        nc.vector.tensor_reduce(o, v, ax, alu.max)

    with tile.TileContext(nc) as tc:
        with tc.tile_pool(name="psum", bufs=1, space="PSUM") as pp, \
                tc.tile_pool(name="dram", bufs=1, space="DRAM") as dram:
            rowboth = pp.tile([R, 2 * T], f32, name="rowboth")
            psum_s = pp.tile([2 * T, 1], f32, name="psum_s")
            meanT_p = pp.tile([1, T], f32, name="meanT_p")
            mr_p = pp.tile([T, T], f32, name="mr_p")
            pl2 = pp.tile([1, 3], f32, name="pl2")

            cstatS = dram.tile([1, 2 * T], f32, name="cstatS")
            cgathS = dram.tile([1, 2 * T], f32, name="cgathS")
            cstatX = dram.tile([1, 2 * T], f32, name="cstatX")
            cgathX = dram.tile([1, 2 * T], f32, name="cgathX")
            cstatG = dram.tile([1, 2], f32, name="cstatG")
            cgathG = dram.tile([1, 2], f32, name="cgathG")

            def allred(cstat, cgath, op, q):
                if single_core or mock_cc:
                    q.dma_start(out=cgath[:], in_=cstat[:])
                else:
                    nc.gpsimd.collective_compute(
                        "AllReduce", op,
                        replica_groups=[list(range(NCORES))],
                        ins=[cstat[:]], outs=[cgath[:]])

            for _rep in range(reps):
                # ---- input loads: I in 4 chunks (2 queues), dsT+cold, cst --
                c2 = CH1 + 1024
                c3 = c2 + 1024
                nc.sync.dma_start(out=I0[:, 0:CH1], in_=dinH[:, 0:CH1])
                nc.scalar.dma_start(out=I0[:, CH1:c2], in_=dinH[:, CH1:c2])
                nc.sync.dma_start(out=I0[:, c2:c3], in_=dinH[:, c2:c3])
                nc.scalar.dma_start(out=I0[:, c3:NSLOT],
                                    in_=dinH[:, c3:NSLOT])
                nc.sync.dma_start(out=dsTC[:], in_=dinH[:, NSLOT:DINH_W])
                nc.scalar.dma_start(out=cstS[:], in_=cst[:])

                # ---- ACT function-table preloads (hidden under input DMA) --
                nc.vector.memset(dummy[0:1, 0:1], 0.0)
                nc.scalar.activation(dummy[0:1, 1:2], dummy[0:1, 0:1],
                                     AF.Square)
                nc.scalar.activation(dummy[0:1, 2:3], dummy[0:1, 0:1],
                                     AF.Sqrt)
                nc.scalar.activation(dummy[0:1, 3:4], dummy[0:1, 0:1],
                                     AF.Relu)
                nc.scalar.copy(dummy[0:1, 4:5], dummy[0:1, 0:1])

                # ---- interleaved sliding max tables (all DVE; level 1
                # chases the input chunks) ----
                nc.vector.tensor_tensor(L2[:, 0:CH1 - 2], I0[:, 0:CH1 - 2],
                                        I0[:, 2:CH1], alu.max)
                nc.vector.tensor_tensor(L2[:, CH1 - 2:c2 - 2],
                                        I0[:, CH1 - 2:c2 - 2],
                                        I0[:, CH1:c2], alu.max)
                nc.vector.tensor_tensor(L2[:, c2 - 2:c3 - 2],
                                        I0[:, c2 - 2:c3 - 2],
                                        I0[:, c2:c3], alu.max)
                nc.vector.tensor_tensor(L2[:, c3 - 2:NSLOT - 2],
                                        I0[:, c3 - 2:NSLOT - 2],
                                        I0[:, c3:NSLOT], alu.max)
                nc.vector.tensor_tensor(L4[:, 0:c3 - 6], L2[:, 0:c3 - 6],
                                        L2[:, 4:c3 - 2], alu.max)
                nc.vector.tensor_tensor(L4[:, c3 - 6:NSLOT - 6],
                                        L2[:, c3 - 6:NSLOT - 6],
                                        L2[:, c3 - 2:NSLOT - 2], alu.max)
                nc.vector.tensor_tensor(L8[:, 0:NSLOT - 14],
                                        L4[:, 0:NSLOT - 14],
                                        L4[:, 8:NSLOT - 6], alu.max)
                nc.vector.tensor_tensor(L16[:, 0:NSLOT - 30],
                                        L8[:, 0:NSLOT - 30],
                                        L8[:, 16:NSLOT - 14], alu.max)
                # global per-row (max | -min) from L16 (early, so the whole
                # wave-1 chain clears HWDGE during the lookups)
                gv = AP(L16.tensor, L16.offset, [list(L16.ap[0]),
                                                 [1, 2], [32, 128]])
                nc.vector.tensor_reduce(gfix[:], gv, X, alu.max)
                nc.vector.tensor_tensor(L32[:, lo32:hi32],
                                        L16[:, lo32:hi32],
                                        L16[:, lo32 + 32:hi32 + 32], alu.max)

                # ---- PE sums path (square on ACT, matmuls on PE) ----
                nc.scalar.square(dsq[:], dsTC[:, 0:W])
                for k in range(NCHUNK):
                    nc.tensor.matmul(rowboth[:, 0:T],
                                     dsTC[:, 128 * k:128 * (k + 1)],
                                     dsTC[:, W + 32 * k:W + 32 * (k + 1)],
                                     start=(k == 0), stop=(k == NCHUNK - 1))
                for k in range(NCHUNK):
                    nc.tensor.matmul(rowboth[:, T:2 * T],
                                     dsq[:, 128 * k:128 * (k + 1)],
                                     dsTC[:, W + 32 * k:W + 32 * (k + 1)],
                                     start=(k == 0), stop=(k == NCHUNK - 1))

                # mask rows (one fused op), reduce to per-box sums on PE
                nc.vector.tensor_tensor(rrs[:], rowboth[:], rind2C, alu.mult)
                nc.tensor.matmul(psum_s[:], rrs[:], onesC,
                                 start=True, stop=True)
                nc.scalar.copy(svS[:], psum_s[:])
                # core-local global (max | -min) across partitions on Pool
                nc.gpsimd.partition_all_reduce(ggP[:], gfix[:], 128, RO.max)
                nc.scalar.dma_start(out=cstatS[0:1, :], in_=svS[:])
                nc.scalar.dma_start(out=cstatG[0:1, :], in_=ggP[0:1, 0:2])
                allred(cstatS, cgathS, alu.add, nc.scalar)
                allred(cstatG, cgathG, alu.max, nc.scalar)

                # ---- fused per-box lookups ----
                for t in range(T):
                    box_lookup(t, int(x1s[t]), int(x2s[t]))
                # row mask (DVE, right after the last lookup) + Pool reduce,
                # feeding the wave-2 tail chain on the SP queue
                nc.vector.tensor_tensor(stk[:], rmm[:], rinfC, alu.add)
                nc.gpsimd.partition_all_reduce(stkP[:], stk[:], 128, RO.max)
                nc.sync.dma_start(out=cstatX[0:1, :], in_=stkP[0:1, 0:2 * T])
                allred(cstatX, cgathX, alu.max, nc.sync)

                # ---- wave-1 readback + mean / loss_acc pipeline ----
                nc.scalar.dma_start(out=sv2[:],
                                    in_=cgathS[0:1, :].transpose([1, 0]))
                nc.scalar.dma_start(out=ggm[:], in_=cgathG[0:1, :])
                nc.vector.tensor_tensor(aden[:], ggm[:, 0:1], ggm[:, 1:2],
                                        alu.add)
                nc.vector.reciprocal(arecip[:], aden[:])
                nc.gpsimd.partition_broadcast(acolS[:], arecip[:])
                nc.vector.tensor_scalar_mul(meanv[:], sv2[0:T, 0:1], cntinvC)
                nc.vector.tensor_scalar_mul(m2sv[:], sv2[0:T, 0:1], meanv[:])
                nc.vector.tensor_scalar(varv[:], sv2[T:2 * T, 0:1], m2sv[:],
                                        cm1invC, alu.subtract, alu.mult)
                nc.scalar.sqrt(stdv[:], varv[:])
                nc.tensor.transpose(meanT_p[:], meanv[:], ident32C)
                nc.scalar.copy(meanTS[:], meanT_p[:])
                nc.tensor.matmul(mr_p[:], onesrowC, meanTS[:],
                                 start=True, stop=True)
                nc.vector.tensor_scalar(qm[:], mr_p[:], meanv[:], acolS[:],
                                        alu.subtract, alu.mult)
                nc.vector.tensor_tensor(t2m[:], gmatC, qm[:], alu.subtract)
                nc.scalar.activation(t3m[:], t2m[:], AF.Relu,
                                     accum_out=raccv[:])
                nc.tensor.matmul(pl2[:, 0:1], raccv[:], cstS[0:T, 66:67],
                                 start=True, stop=True)

                # ---- wave-2 readback + final (SP queue, DVE math) ----
                # redM rows 0:32 = global max_t, rows 32:64 = global -min_t
                # (block layout straight from the lookups; AllReduce'd)
                nc.sync.dma_start(out=redM[:],
                                  in_=cgathX[0:1, :].transpose([1, 0]))
                nc.vector.tensor_scalar_add(rngv[:], redM[T:2 * T, 0:1],
                                            redM[0:T, 0:1])
                nc.vector.reciprocal(rinv[:], rngv[:])
                nc.vector.tensor_tensor(srv[:], stdv[:], rinv[:], alu.mult)
                nc.tensor.matmul(pl2[:, 1:2], srv[:], cstS[0:T, 66:67],
                                 start=True, stop=True)
                nc.vector.tensor_scalar(out3[:, 0:2], pl2[:, 0:2], 0.0, 0.0,
                                        alu.add, alu.add,
                                        accum_out=out3[:, 2:3])
                nc.sync.dma_start(out=out[:], in_=out3[0:1, 0:3])

    nc.compile()
    return nc


def kernel(d_pred, bboxes, _trace=False):
    from concourse.bass_utils import run_bass_kernel_spmd

    d_pred = np.asarray(d_pred, dtype=np.float32)
    bboxes = np.asarray(bboxes, dtype=np.int32)
    depth = d_pred[0, 0]
    x1, y1, x2, y2 = (bboxes[:, i].astype(np.int64) for i in range(4))

    cnt = ((x2 - x1) * (y2 - y1)).astype(np.float64)
    cntinv = (1.0 / cnt).astype(np.float32)
    cm1inv = (1.0 / (cnt - 1.0)).astype(np.float32)

    ii = np.arange(T)[:, None]
    jj = np.arange(T)[None, :]
    gmat = np.where(jj > ii, (jj - ii) / float(T), -BIG).astype(np.float32)

    rows = np.arange(H)
    rind_full = ((rows[:, None] >= y1[None, :])
                 & (rows[:, None] < y2[None, :])).astype(np.float32)

    cols = np.arange(W)
    colind_full = ((cols[:, None] >= x1[None, :])
                   & (cols[:, None] < x2[None, :])).astype(np.float16)
    cold_h = colind_full.reshape(NCHUNK, 128, T).transpose(1, 0, 2) \
        .reshape(128, NCHUNK * T)

    in_maps = []
    for c in range(NCORES):
        dloc = depth[c * R:(c + 1) * R]                       # [128, 2048]
        ri = rind_full[c * R:(c + 1) * R]                     # [128, 32]
        rinf = np.where(ri > 0, 0.0, -BIG).astype(np.float32)

        cstc = np.zeros((128, CST_W), np.float32)
        cstc[0:T, 0:T] = np.eye(T, dtype=np.float32)
        cstc[0:T, 32:64] = gmat
        cstc[0:T, 64] = cntinv
        cstc[0:T, 65] = cm1inv
        cstc[:, 66] = 1.0
        cstc[0, 68:68 + T] = 1.0
        # block row mask: cols 0:32 max side, 32:64 -min side (same mask)
        cstc[:, 100:132] = rinf
        cstc[:, 132:164] = rinf
        cstc[:, 164:196] = ri
        cstc[:, 196:228] = ri

        dsT_h = dloc.T.reshape(NCHUNK, 128, R).transpose(1, 0, 2) \
            .reshape(128, W).astype(np.float16)

        dinH = np.empty((R, DINH_W), np.float16)
        dinH[:, 0:NSLOT:2] = dloc.astype(np.float16)
        dinH[:, 1:NSLOT:2] = (-dloc).astype(np.float16)
        dinH[:, NSLOT:NSLOT + W] = dsT_h
        dinH[:, NSLOT + W:DINH_W] = cold_h
        in_maps.append({"dinH": dinH, "cst": cstc})

    nc = _build_program(bboxes)
    res = run_bass_kernel_spmd(nc, in_maps, list(range(NCORES)),
                               trace=_trace)
    o = res.results[0]["out"].astype(np.float32)
    outs = (o[0:1].copy(), o[1:2].copy(), o[2:3].copy())
    if _trace:
        return outs, res
    return outs
